# revision 1
# baseline (speedup 1.0000x reference)
"""MoE layer (T=8192, d=1024, dff=1024, E=64, top-k=2, capacity factor 2)
on 8 Trainium2 NeuronCores, expert-parallel.

Strategy
--------
Host (cheap, O(N) index math + gathers):
  * compute each expanded token's expert and its position within the expert
    (the reference's cumsum-over-one-hot routing), applying the capacity cap
  * experts are sharded 8-per-core; tokens routed to an expert are packed
    into a [d, cols] column block for that expert, TRANSPOSED and cast to
    bf16 so the device needs no on-chip transposes
  * per-expert column counts are padded to the max across cores so all 8
    cores run one identical SPMD program (shapes baked at build time)

Device (one Bass/Tile program, built for the observed count vector):
  * per expert e: h_T = W1_e^T @ x_T  (PSUM, fp32 accum)
                  act_T = silu(gate_T) * up_T   (ACT + DVE, cast bf16)
                  y_T  = W2_e^T @ act_T          (PSUM, fp32 accum)
  * weights stream HBM->SBUF double-buffered, one large DMA per tensor per
    expert, issued in exactly the order the PE consumes them (the sync-engine
    HWDGE ring drains roughly FIFO at full HBM bandwidth); x_T stays resident;
    y stored bf16 on the scalar engine's separate DMA ring
  * expert 0's w1 arrives in per-k-tile chunks and is processed k-outer so
    its matmuls trickle-start at ~4us, doubling as PE clock-gate warm-up

Host combine: gather each expanded token's output column, weight by router
prob, sum over the k=2 copies.
"""

import numpy as np
import ml_dtypes

from concourse import bacc, mybir
import concourse.tile as tile
from concourse.bass_utils import run_bass_kernel_spmd

P = 128
NCORES = 8

BF16 = ml_dtypes.bfloat16

_program_cache: dict = {}


def _build_program(cnts: tuple, d: int, dff: int, epc: int):
    """Build+compile the SPMD Bass program for per-slot column counts `cnts`."""
    ctot = sum(cnts)
    f32 = mybir.dt.float32
    bf = mybir.dt.bfloat16

    nc = bacc.Bacc("TRN2", target_bir_lowering=False, debug=False)
    xT = nc.declare_dram_parameter("xT", [d, ctot], bf, isOutput=False)
    gup = nc.declare_dram_parameter("gup", [epc, d, 2 * dff], bf, isOutput=False)
    dn = nc.declare_dram_parameter("dn", [epc, dff, d], bf, isOutput=False)
    y = nc.declare_dram_parameter("y", [d, ctot], bf, isOutput=True)
    OT = d // P

    KT = d // P    # contraction tiles for mm1
    FT = dff // P  # dff tiles (rows of h_T per gate/up half)
    nmax = max(cnts)

    xT3 = xT.rearrange("(kk p) c -> p kk c", p=P)      # [P, KT, ctot]
    y3 = y.rearrange("(oi p) c -> p oi c", p=P)        # [P, OT, ctot]

    with tile.TileContext(nc) as tc:
        with (
            tc.tile_pool(name="xpool", bufs=1) as xpool,
            tc.tile_pool(name="w1pool", bufs=2) as w1pool,
            tc.tile_pool(name="w2pool", bufs=2) as w2pool,
            tc.tile_pool(name="actpool", bufs=2) as actpool,
            tc.tile_pool(name="ypool", bufs=3) as ypool,
            tc.tile_pool(name="evict", bufs=3) as evict,
            tc.tile_pool(name="ps1", bufs=2, space="PSUM") as ps1,
            tc.tile_pool(name="ps2", bufs=2, space="PSUM") as ps2,
        ):
            # The sync-engine HWDGE ring drains DMAs roughly in issue order at
            # full HBM bandwidth, so DMAs are issued in exactly the order the
            # PE consumes them: expert 0's token block, then its w1 k-chunks
            # (its matmuls trickle-start at ~3us and double as PE warm-up),
            # then per expert j: token block, w1, w2.
            xt = xpool.tile([P, KT, ctot], bf, tag="xt")

            def mm1_swiglu(w1, act, off, nj, i):
                gate_ps = ps1.tile([P, nj], f32, tag="gate")
                up_ps = ps1.tile([P, nj], f32, tag="up")
                for kk in range(KT):
                    nc.tensor.matmul(
                        gate_ps[:],
                        lhsT=w1[:, kk, i * P : (i + 1) * P],
                        rhs=xt[:, kk, off : off + nj],
                        start=(kk == 0),
                        stop=(kk == KT - 1),
                    )
                for kk in range(KT):
                    nc.tensor.matmul(
                        up_ps[:],
                        lhsT=w1[:, kk, dff + i * P : dff + (i + 1) * P],
                        rhs=xt[:, kk, off : off + nj],
                        start=(kk == 0),
                        stop=(kk == KT - 1),
                    )
                silu_sb = evict.tile([P, nj], f32, tag="silu")
                nc.scalar.activation(
                    silu_sb[:], gate_ps[:], mybir.ActivationFunctionType.Silu
                )
                nc.vector.tensor_mul(act[:, i, :nj], silu_sb[:], up_ps[:])

            # ~6us of throwaway matmuls bridge the PE clock-gate's busy window
            # from t=0 until the first real (DMA-gated) matmul, so the real
            # stream starts at 2.4GHz instead of ramping from 1.2.
            scratch = evict.tile([P, 512], bf, tag="scratch")
            nc.vector.memset(scratch[:], 0.0)
            junk_ps = ps2.tile([P, 512], f32, tag="junk")
            for _ in range(28):
                nc.tensor.matmul(
                    junk_ps[:], lhsT=scratch[:, :P], rhs=scratch[:],
                    start=True, stop=True,
                )

            off = 0
            for j in range(epc):
                nj = cnts[j]
                if nj == 0:
                    continue
                nc.sync.dma_start(xt[:, :, off : off + nj], xT3[:, :, off : off + nj])
                w1 = w1pool.tile([P, KT, 2 * dff], bf, tag="w1")
                gup3 = gup[j].rearrange("(kk p) c -> p kk c", p=P)
                if j == 0:
                    # chunked so the first matmuls are eligible after ~1.2MB
                    for kk in range(KT):
                        nc.sync.dma_start(w1[:, kk, :], gup3[:, kk, :])
                else:
                    nc.sync.dma_start(w1[:], gup3[:])
                w2 = w2pool.tile([P, FT, d], bf, tag="w2")
                nc.sync.dma_start(w2[:], dn[j].rearrange("(kk p) c -> p kk c", p=P))

                act = actpool.tile([P, FT, nmax], bf, tag="act")
                if j == 0:
                    # k-outer over pairs of i-blocks: matmuls become eligible
                    # chunk-by-chunk while x/w1 stream in (4 PSUM banks).
                    for i0 in range(0, FT, 2):
                        g0 = ps1.tile([P, nj], f32, tag="gate")
                        u0 = ps1.tile([P, nj], f32, tag="up")
                        g1 = ps1.tile([P, nj], f32, tag="gate")
                        u1 = ps1.tile([P, nj], f32, tag="up")
                        for kk in range(KT):
                            fl = {"start": kk == 0, "stop": kk == KT - 1}
                            x_ap = xt[:, kk, off : off + nj]
                            for di, (g, u) in enumerate(((g0, u0), (g1, u1))):
                                i = i0 + di
                                nc.tensor.matmul(
                                    g[:], lhsT=w1[:, kk, i * P : (i + 1) * P],
                                    rhs=x_ap, **fl,
                                )
                                nc.tensor.matmul(
                                    u[:],
                                    lhsT=w1[:, kk, dff + i * P : dff + (i + 1) * P],
                                    rhs=x_ap, **fl,
                                )
                        for di, (g, u) in enumerate(((g0, u0), (g1, u1))):
                            i = i0 + di
                            silu_sb = evict.tile([P, nj], f32, tag="silu")
                            nc.scalar.activation(
                                silu_sb[:], g[:], mybir.ActivationFunctionType.Silu
                            )
                            nc.vector.tensor_mul(act[:, i, :nj], silu_sb[:], u[:])
                else:
                    for i in range(FT):
                        mm1_swiglu(w1, act, off, nj, i)

                yt = ypool.tile([P, OT, nj], bf, tag="yt")
                for oi in range(OT):
                    y_ps = ps2.tile([P, nj], f32, tag="y")
                    for kk in range(FT):
                        nc.tensor.matmul(
                            y_ps[:],
                            lhsT=w2[:, kk, oi * P : (oi + 1) * P],
                            rhs=act[:, kk, :nj],
                            start=(kk == 0),
                            stop=(kk == FT - 1),
                        )
                    nc.vector.tensor_copy(yt[:, oi, :], y_ps[:])
                # output goes out on the scalar engine's separate HWDGE ring
                # so stores never delay the weight-read stream
                nc.scalar.dma_start(y3[:, :, off : off + nj], yt[:])
                off += nj

    nc.compile()
    return nc


def _route(topk_indices: np.ndarray, E: int, C: int):
    """Reference-equivalent routing: per expanded token, its within-expert
    position in flat (t, k) order; tokens beyond capacity C are dropped."""
    e = np.asarray(topk_indices).reshape(-1).astype(np.int64)
    N = e.shape[0]
    order = np.argsort(e, kind="stable")  # grouped by expert, flat order kept
    counts = np.bincount(e, minlength=E)
    starts = np.zeros(E + 1, np.int64)
    np.cumsum(counts, out=starts[1:])
    rank = np.arange(N, dtype=np.int64) - starts[e[order]]  # pos within expert
    pos = np.empty(N, np.int64)
    pos[order] = rank
    keep = pos < C
    return e, pos, keep, counts


def kernel(
    hidden_states: np.ndarray,
    topk_indices: np.ndarray,
    topk_weights: np.ndarray,
    gate_up_proj: np.ndarray,
    down_proj: np.ndarray,
) -> np.ndarray:
    hs = np.asarray(hidden_states, dtype=np.float32)
    tw = np.asarray(topk_weights, dtype=np.float32)
    gupw = np.asarray(gate_up_proj, dtype=np.float32)
    dnw = np.asarray(down_proj, dtype=np.float32)

    T, d = hs.shape
    k = np.asarray(topk_indices).shape[-1]
    E, _, dff2 = gupw.shape
    dff = dff2 // 2
    N = T * k
    C = (2 * N) // E  # CAPACITY_FACTOR = 2
    epc = E // NCORES

    e, pos, keep, _ = _route(topk_indices, E, C)
    posc = np.minimum(pos, C - 1)
    kept_idx = np.where(keep)[0]

    # Kept-token count per expert, then deal experts to (slot, core) by global
    # rank: slot j on every core holds the experts ranked 8j..8j+7 by count.
    # Per-slot counts are then nearly equal across cores, so the SPMD padding
    # (max over cores) wastes ~1% instead of ~10%. Slot 0 is the biggest
    # (overlaps the startup DMA trickle), the last slot smallest (short tail).
    ce = np.bincount(e[kept_idx], minlength=E)
    order = np.argsort(-ce, kind="stable")  # experts by count, descending
    assign = order.reshape(epc, NCORES)  # [slot j, core m] -> expert id
    core_of_expert = np.empty(E, np.int64)
    slot_of_expert = np.empty(E, np.int64)
    for j in range(epc):
        for m in range(NCORES):
            core_of_expert[assign[j, m]] = m
            slot_of_expert[assign[j, m]] = j

    cnts = tuple(
        int(-(-max(int(ce[assign[j]].max()), 1) // 4) * 4) for j in range(epc)
    )
    ctot = sum(cnts)
    offs_prog = np.zeros(epc, np.int64)
    np.cumsum(np.asarray(cnts[:-1], np.int64), out=offs_prog[1:])

    core_of = core_of_expert[e]  # per expanded token
    # column of each kept expanded token inside its core's packed layout
    col = offs_prog[slot_of_expert[e]] + posc  # valid where keep

    key = (cnts, d, dff, epc)
    nc = _program_cache.get(key)
    if nc is None:
        nc = _build_program(cnts, d, dff, epc)
        _program_cache[key] = nc

    hsb = hs.astype(BF16)
    tok_of_n = np.arange(N, dtype=np.int64) // k

    in_maps = []
    for m in range(NCORES):
        X = np.zeros((ctot, d), BF16)
        sel = kept_idx[core_of[kept_idx] == m]
        X[col[sel]] = hsb[tok_of_n[sel]]
        eids = assign[:, m]  # this core's experts in program (slot) order
        in_maps.append(
            {
                "xT": np.ascontiguousarray(X.T),
                "gup": np.ascontiguousarray(gupw[eids]).astype(BF16),
                "dn": np.ascontiguousarray(dnw[eids]).astype(BF16),
            }
        )

    res = run_bass_kernel_spmd(nc, in_maps, list(range(NCORES)))

    # combine: rows[n] = y_core(n)[:, col(n)] for kept n, 0 otherwise
    rows = np.zeros((N, d), np.float32)
    for m in range(NCORES):
        Ym = np.asarray(res.results[m]["y"]).astype(np.float32)  # [d, ctot]
        sel = kept_idx[core_of[kept_idx] == m]
        rows[sel] = Ym.T[col[sel]]
    wf = tw.reshape(-1) * keep.astype(np.float32)
    out = (rows * wf[:, None]).reshape(T, k, d).sum(axis=1)
    return out.astype(hs.dtype)



# revision 8
# speedup vs baseline: 1.0303x; 1.0303x over previous
"""MoE layer (T=8192, d=1024, dff=1024, E=64, top-k=2, capacity factor 2)
on 8 Trainium2 NeuronCores, expert-parallel.

Strategy
--------
Host (cheap, O(N) index math + gathers):
  * compute each expanded token's expert and its position within the expert
    (the reference's cumsum-over-one-hot routing), applying the capacity cap
  * experts are sharded 8-per-core; tokens routed to an expert are packed
    into a [d, cols] column block for that expert, TRANSPOSED and cast to
    bf16 so the device needs no on-chip transposes
  * per-expert column counts are padded to the max across cores so all 8
    cores run one identical SPMD program (shapes baked at build time)

Device (one Bass/Tile program, built for the observed count vector):
  * per expert e: h_T = W1_e^T @ x_T  (PSUM, fp32 accum)
                  act_T = silu(gate_T) * up_T   (ACT + DVE, cast bf16)
                  y_T  = W2_e^T @ act_T          (PSUM, fp32 accum)
  * weights stream HBM->SBUF double-buffered, one large DMA per tensor per
    expert, issued in exactly the order the PE consumes them (the sync-engine
    HWDGE ring drains roughly FIFO at full HBM bandwidth); x_T stays resident;
    y stored bf16 on the scalar engine's separate DMA ring
  * expert 0's w1 arrives in per-k-tile chunks and is processed k-outer so
    its matmuls trickle-start at ~4us, doubling as PE clock-gate warm-up

Host combine: gather each expanded token's output column, weight by router
prob, sum over the k=2 copies.
"""

import numpy as np
import ml_dtypes

from concourse import bacc, mybir
import concourse.tile as tile
from concourse.bass_utils import run_bass_kernel_spmd

P = 128
NCORES = 8

BF16 = ml_dtypes.bfloat16

_program_cache: dict = {}


def _build_program(cnts: tuple, d: int, dff: int, epc: int):
    """Build+compile the SPMD Bass program for per-slot column counts `cnts`."""
    ctot = sum(cnts)
    f32 = mybir.dt.float32
    bf = mybir.dt.bfloat16

    nc = bacc.Bacc("TRN2", target_bir_lowering=False, debug=False)
    xT = nc.declare_dram_parameter("xT", [d, ctot], bf, isOutput=False)
    gup = nc.declare_dram_parameter("gup", [epc, d, 2 * dff], bf, isOutput=False)
    dn = nc.declare_dram_parameter("dn", [epc, dff, d], bf, isOutput=False)
    y = nc.declare_dram_parameter("y", [d, ctot], bf, isOutput=True)
    OT = d // P

    KT = d // P    # contraction tiles for mm1
    FT = dff // P  # dff tiles (rows of h_T per gate/up half)
    nmax = max(cnts)

    xT3 = xT.rearrange("(kk p) c -> p kk c", p=P)      # [P, KT, ctot]
    y3 = y.rearrange("(oi p) c -> p oi c", p=P)        # [P, OT, ctot]

    with tile.TileContext(nc) as tc:
        with (
            tc.tile_pool(name="xpool", bufs=1) as xpool,
            tc.tile_pool(name="w1pool", bufs=2) as w1pool,
            tc.tile_pool(name="w2pool", bufs=2) as w2pool,
            tc.tile_pool(name="actpool", bufs=2) as actpool,
            tc.tile_pool(name="ypool", bufs=3) as ypool,
            tc.tile_pool(name="evict", bufs=3) as evict,
            tc.tile_pool(name="ps1", bufs=2, space="PSUM") as ps1,
            tc.tile_pool(name="ps2", bufs=2, space="PSUM") as ps2,
        ):
            # The sync-engine HWDGE ring drains DMAs roughly in issue order at
            # full HBM bandwidth, so DMAs are issued in exactly the order the
            # PE consumes them: expert 0's token block, then its w1 k-chunks
            # (its matmuls trickle-start at ~3us and double as PE warm-up),
            # then per expert j: token block, w1, w2.
            xt = xpool.tile([P, KT, ctot], bf, tag="xt")

            def mm1_swiglu(w1, act, off, nj, i):
                gate_ps = ps1.tile([P, nj], f32, tag="gate")
                up_ps = ps1.tile([P, nj], f32, tag="up")
                for kk in range(KT):
                    nc.tensor.matmul(
                        gate_ps[:],
                        lhsT=w1[:, kk, i * P : (i + 1) * P],
                        rhs=xt[:, kk, off : off + nj],
                        start=(kk == 0),
                        stop=(kk == KT - 1),
                    )
                for kk in range(KT):
                    nc.tensor.matmul(
                        up_ps[:],
                        lhsT=w1[:, kk, dff + i * P : dff + (i + 1) * P],
                        rhs=xt[:, kk, off : off + nj],
                        start=(kk == 0),
                        stop=(kk == KT - 1),
                    )
                silu_sb = evict.tile([P, nj], f32, tag="silu")
                nc.scalar.activation(
                    silu_sb[:], gate_ps[:], mybir.ActivationFunctionType.Silu
                )
                nc.vector.tensor_mul(act[:, i, :nj], silu_sb[:], up_ps[:])

            # ~5us of throwaway matmuls bridge the PE clock-gate's busy window
            # from t=0 until the first real (DMA-gated) matmul, so the real
            # stream starts at 2.4GHz instead of ramping from 1.2.
            scratch = evict.tile([P, 512], bf, tag="scratch")
            nc.vector.memset(scratch[:], 0.0)
            junk_ps = ps2.tile([P, 512], f32, tag="junk")
            for _ in range(22):
                nc.tensor.matmul(
                    junk_ps[:], lhsT=scratch[:, :P], rhs=scratch[:],
                    start=True, stop=True,
                )

            off = 0
            for j in range(epc):
                nj = cnts[j]
                if nj == 0:
                    continue
                if j == 0:
                    # expert 0's token block rides the scalar engine's (empty)
                    # HWDGE ring so its descriptor-gen overlaps w1 chunk 0's
                    # on the sync ring: first matmul eligible ~0.7us earlier
                    nc.scalar.dma_start(
                        xt[:, :, off : off + nj], xT3[:, :, off : off + nj]
                    )
                else:
                    nc.sync.dma_start(
                        xt[:, :, off : off + nj], xT3[:, :, off : off + nj]
                    )
                w1 = w1pool.tile([P, KT, 2 * dff], bf, tag="w1")
                gup3 = gup[j].rearrange("(kk p) c -> p kk c", p=P)
                if j == 0:
                    # chunked so the first matmuls are eligible after ~1.2MB
                    for kk in range(KT):
                        nc.sync.dma_start(w1[:, kk, :], gup3[:, kk, :])
                else:
                    nc.sync.dma_start(w1[:], gup3[:])
                w2 = w2pool.tile([P, FT, d], bf, tag="w2")
                nc.sync.dma_start(w2[:], dn[j].rearrange("(kk p) c -> p kk c", p=P))

                act = actpool.tile([P, FT, nmax], bf, tag="act")
                if j == 0:
                    # k-outer over pairs of i-blocks: matmuls become eligible
                    # chunk-by-chunk while x/w1 stream in (4 PSUM banks).
                    for i0 in range(0, FT, 2):
                        g0 = ps1.tile([P, nj], f32, tag="gate")
                        u0 = ps1.tile([P, nj], f32, tag="up")
                        g1 = ps1.tile([P, nj], f32, tag="gate")
                        u1 = ps1.tile([P, nj], f32, tag="up")
                        for kk in range(KT):
                            fl = {"start": kk == 0, "stop": kk == KT - 1}
                            x_ap = xt[:, kk, off : off + nj]
                            for di, (g, u) in enumerate(((g0, u0), (g1, u1))):
                                i = i0 + di
                                nc.tensor.matmul(
                                    g[:], lhsT=w1[:, kk, i * P : (i + 1) * P],
                                    rhs=x_ap, **fl,
                                )
                                nc.tensor.matmul(
                                    u[:],
                                    lhsT=w1[:, kk, dff + i * P : dff + (i + 1) * P],
                                    rhs=x_ap, **fl,
                                )
                        for di, (g, u) in enumerate(((g0, u0), (g1, u1))):
                            i = i0 + di
                            silu_sb = evict.tile([P, nj], f32, tag="silu")
                            nc.scalar.activation(
                                silu_sb[:], g[:], mybir.ActivationFunctionType.Silu
                            )
                            nc.vector.tensor_mul(act[:, i, :nj], silu_sb[:], u[:])
                else:
                    for i in range(FT):
                        mm1_swiglu(w1, act, off, nj, i)

                yt = ypool.tile([P, OT, nj], bf, tag="yt")
                last = j == epc - 1
                for oi in range(OT):
                    y_ps = ps2.tile([P, nj], f32, tag="y")
                    for kk in range(FT):
                        nc.tensor.matmul(
                            y_ps[:],
                            lhsT=w2[:, kk, oi * P : (oi + 1) * P],
                            rhs=act[:, kk, :nj],
                            start=(kk == 0),
                            stop=(kk == FT - 1),
                        )
                    nc.vector.tensor_copy(yt[:, oi, :], y_ps[:])
                    if last:
                        # tail: store per-oi so the final DMA trails the last
                        # matmul by ~0.2us instead of a whole-expert store
                        nc.scalar.dma_start(
                            y3[:, oi, off : off + nj], yt[:, oi, :]
                        )
                # output goes out on the scalar engine's separate HWDGE ring
                # so stores never delay the weight-read stream
                if not last:
                    nc.scalar.dma_start(y3[:, :, off : off + nj], yt[:])
                off += nj

    nc.compile()
    return nc


def _route(topk_indices: np.ndarray, E: int, C: int):
    """Reference-equivalent routing: per expanded token, its within-expert
    position in flat (t, k) order; tokens beyond capacity C are dropped."""
    e = np.asarray(topk_indices).reshape(-1).astype(np.int64)
    N = e.shape[0]
    order = np.argsort(e, kind="stable")  # grouped by expert, flat order kept
    counts = np.bincount(e, minlength=E)
    starts = np.zeros(E + 1, np.int64)
    np.cumsum(counts, out=starts[1:])
    rank = np.arange(N, dtype=np.int64) - starts[e[order]]  # pos within expert
    pos = np.empty(N, np.int64)
    pos[order] = rank
    keep = pos < C
    return e, pos, keep, counts


def kernel(
    hidden_states: np.ndarray,
    topk_indices: np.ndarray,
    topk_weights: np.ndarray,
    gate_up_proj: np.ndarray,
    down_proj: np.ndarray,
) -> np.ndarray:
    hs = np.asarray(hidden_states, dtype=np.float32)
    tw = np.asarray(topk_weights, dtype=np.float32)
    gupw = np.asarray(gate_up_proj, dtype=np.float32)
    dnw = np.asarray(down_proj, dtype=np.float32)

    T, d = hs.shape
    k = np.asarray(topk_indices).shape[-1]
    E, _, dff2 = gupw.shape
    dff = dff2 // 2
    N = T * k
    C = (2 * N) // E  # CAPACITY_FACTOR = 2
    epc = E // NCORES

    e, pos, keep, _ = _route(topk_indices, E, C)
    posc = np.minimum(pos, C - 1)

    # Tokens whose top-2 experts coincide compute the same expert output
    # twice; drop the second copy and fold its router weight into the first.
    # Safe while no expert is near capacity (counts max out far below C).
    wf_eff = tw.reshape(-1).copy()
    dup = (e[0::2] == e[1::2]) & keep[0::2] & keep[1::2] if k == 2 else np.zeros(0, bool)
    if dup.any():
        di = np.where(dup)[0]
        wf_eff[2 * di] += wf_eff[2 * di + 1]
        keep = keep.copy()
        keep[2 * di + 1] = False
    kept_idx = np.where(keep)[0]

    # Kept-token count per expert, then deal experts to (slot, core) by global
    # rank: slot j on every core holds the experts ranked 8j..8j+7 by count.
    # Per-slot counts are then nearly equal across cores, so the SPMD padding
    # (max over cores) wastes ~1% instead of ~10%. Slot 0 is the biggest
    # (overlaps the startup DMA trickle), the last slot smallest (short tail).
    ce = np.bincount(e[kept_idx], minlength=E)
    order = np.argsort(-ce, kind="stable")  # experts by count, descending
    assign = order.reshape(epc, NCORES)  # [slot j, core m] -> expert id
    core_of_expert = np.empty(E, np.int64)
    slot_of_expert = np.empty(E, np.int64)
    for j in range(epc):
        for m in range(NCORES):
            core_of_expert[assign[j, m]] = m
            slot_of_expert[assign[j, m]] = j

    cnts = tuple(
        int(-(-max(int(ce[assign[j]].max()), 1) // 2) * 2) for j in range(epc)
    )
    ctot = sum(cnts)
    offs_prog = np.zeros(epc, np.int64)
    np.cumsum(np.asarray(cnts[:-1], np.int64), out=offs_prog[1:])

    core_of = core_of_expert[e]  # per expanded token
    # column of each kept expanded token inside its core's packed layout
    col = offs_prog[slot_of_expert[e]] + posc  # valid where keep

    key = (cnts, d, dff, epc)
    nc = _program_cache.get(key)
    if nc is None:
        nc = _build_program(cnts, d, dff, epc)
        _program_cache[key] = nc

    hsb = hs.astype(BF16)
    tok_of_n = np.arange(N, dtype=np.int64) // k

    in_maps = []
    for m in range(NCORES):
        X = np.zeros((ctot, d), BF16)
        sel = kept_idx[core_of[kept_idx] == m]
        X[col[sel]] = hsb[tok_of_n[sel]]
        eids = assign[:, m]  # this core's experts in program (slot) order
        in_maps.append(
            {
                "xT": np.ascontiguousarray(X.T),
                "gup": np.ascontiguousarray(gupw[eids]).astype(BF16),
                "dn": np.ascontiguousarray(dnw[eids]).astype(BF16),
            }
        )

    res = run_bass_kernel_spmd(nc, in_maps, list(range(NCORES)))

    # combine: rows[n] = y_core(n)[:, col(n)] for kept n, 0 otherwise
    rows = np.zeros((N, d), np.float32)
    for m in range(NCORES):
        Ym = np.asarray(res.results[m]["y"]).astype(np.float32)  # [d, ctot]
        sel = kept_idx[core_of[kept_idx] == m]
        rows[sel] = Ym.T[col[sel]]
    wf = wf_eff * keep.astype(np.float32)
    out = (rows * wf[:, None]).reshape(T, k, d).sum(axis=1)
    return out.astype(hs.dtype)

